# revision 34
# baseline (speedup 1.0000x reference)
"""3-layer GCN (AccessibilityGNN) on 8 Trainium2 NeuronCores.

Strategy (graph/data parallel, per sharding hint):
- Nodes row-sharded 6250/core. Per conv: local GEMM M = H @ W (bf16, PSUM fp32),
  fold dis[src] into M via per-partition ACT scale, AllGather M' (bf16) across
  the 8 cores, then each core aggregates its own dst rows: per 128-dst tile,
  indirect-DMA gather of source rows + one-hot (0/1) matmul accumulation in
  PSUM (S built on device via is_equal against an iota tile). Evacuate with
  dis[dst] scale + bias (+ relu for convs 1-2). H for the next conv's GEMM is
  produced feature-major via PE transposes. Heads = one fused [256,72] GEMM.

Host side: degree/normalization, edge sorting/chunking/padding, input
transpose+casts, output unshard.
"""
import os
import sys

sys.path.insert(0, "/opt/trn_rl_repo")

import numpy as np
import ml_dtypes

N = 50000
E = 800000
D_IN = 512
D_H = 512
D_E = 256
NCORES = 8
NSH = N // NCORES  # 6250
P = 128
NT = (NSH + P - 1) // P  # 49 tiles; last tile has 106 rows
NSHD = NT * P  # device-padded shard rows (6272) so all compute tiles are full

BF16 = ml_dtypes.bfloat16

# toggles (env-overridable for experiments)
# NOTE: multi-index indirect DMA (GB>1) silently gathers wrong rows on HW —
# only the first index column is honored. Keep GB=1.
GB = int(os.environ.get("GCN_GATHER_BATCH", "1"))   # chunks per indirect DMA
SB = int(os.environ.get("GCN_ISEQ_BATCH", "8"))     # chunks per is_equal
TRACE = bool(int(os.environ.get("GCN_TRACE", "0")))
NT_LIM = int(os.environ.get("GCN_NT_LIM", "0")) or None   # debug: limit tiles/conv
NCONVS = int(os.environ.get("GCN_CONVS", "3"))            # debug: number of convs
DG = bool(int(os.environ.get("GCN_DG", "1")))             # use dma_gather batching
DGW = int(os.environ.get("GCN_DGW", "7"))                # chunks per dma_gather
DMASCR = int(os.environ.get("GCN_DMASCR", "16384"))        # SWDGE ring bytes (1024 descs default)
NQ = int(os.environ.get("GCN_NQ", "1"))                    # SWDGE queues for gathers
PRE = bool(int(os.environ.get("GCN_PRE", "0")))            # preload idx table (128B-aligned slices)
HALF = NCORES * NSHD // 2                                  # gather-table half rows


def _dg_groups(C_th):
    """Group split: (tile, half, S-col start, width, aligned idx-col start)."""
    out = []
    gc = 0
    ic = 0
    for t in range(NT):
        for h in range(2):
            c_rem = int(C_th[t][h])
            while c_rem > 0:
                w = min(DGW, c_rem)
                ic = (ic + 7) // 8 * 8  # 8 chunks * 16B = 128B alignment
                out.append((t, h, gc, w, ic))
                gc += w
                ic += w
                c_rem -= w
    return out


def _prep(x, edge_index, W1, b1, W2, b2, W3, b3, We, be, Wh, bh, Wg, bg):
    """Host-side sharding + edge-plan construction."""
    src = edge_index[0].astype(np.int64)
    dst = edge_index[1].astype(np.int64)
    loops = np.arange(N, dtype=np.int64)
    src_all = np.concatenate([src, loops])
    dst_all = np.concatenate([dst, loops])

    deg = np.bincount(dst_all, minlength=N).astype(np.float64)
    dis = (1.0 / np.sqrt(deg)).astype(np.float32)

    order = np.argsort(dst_all, kind="stable")
    s_sorted = src_all[order]
    d_sorted = dst_all[order]

    core_of = d_sorted // NSH
    t_of = (d_sorted % NSH) // P
    gt = core_of * NT + t_of  # global (core,tile) id; monotone since d_sorted sorted
    counts = np.bincount(gt, minlength=NCORES * NT)
    starts = np.concatenate([[0], np.cumsum(counts)])

    # common per-tile chunk count across cores
    cmat = counts.reshape(NCORES, NT)
    C_t = np.maximum(1, (cmat.max(axis=0) + P - 1) // P).astype(np.int64)  # [NT]
    col_base = np.concatenate([[0], np.cumsum(C_t)])
    NCH = int(col_base[-1])

    gidx = np.zeros((NCORES, P, NCH), np.int32)
    dstl = np.full((NCORES, P, NCH), -1.0, np.float32)
    r = np.arange(len(d_sorted)) - starts[gt]
    col = col_base[t_of] + r // P
    row = r % P
    # remap to device-padded row ids: node (c, j) lives at row c*NSHD + j
    r_glob = ((s_sorted // NSH) * NSHD + (s_sorted % NSH)).astype(np.int64)
    gidx[core_of, row, col] = r_glob.astype(np.int32)
    dstl[core_of, row, col] = ((d_sorted % NSH) - t_of * P).astype(np.float32)

    # ---- dma_gather plan: edges regrouped by (core, tile, table-half) ----
    h_of = (r_glob // HALF).astype(np.int64)
    grp = (core_of * NT + t_of) * 2 + h_of
    order2 = np.argsort(grp, kind="stable")
    g2 = grp[order2]
    cnts = np.bincount(g2, minlength=NCORES * NT * 2)
    C_th = np.ceil(cnts.reshape(NCORES, NT, 2).max(axis=0) / P).astype(np.int64)  # [NT,2]
    colbase_th = np.concatenate([[0], np.cumsum(C_th.reshape(-1))]).reshape(-1)
    NCH_DG = int(colbase_th[-1])
    starts2 = np.concatenate([[0], np.cumsum(cnts)])
    rank2 = np.arange(len(g2)) - starts2[g2]
    t2 = t_of[order2]
    h2 = h_of[order2]
    col2 = colbase_th[t2 * 2 + h2] + rank2 // P
    row2 = rank2 % P
    lidx = np.zeros((NCORES, P, NCH_DG), np.int64)   # local (half) row ids, pad=0
    dstl_dg = np.full((NCORES, P, NCH_DG), -1.0, np.float32)
    lidx[core_of[order2], row2, col2] = r_glob[order2] % HALF
    dstl_dg[core_of[order2], row2, col2] = ((d_sorted[order2] % NSH) - t2 * P).astype(np.float32)
    # wrap per chunk: dgidx_w[16*rep + q, col*8 + c2] = lidx[c2*16 + q, col]
    lw = lidx.reshape(NCORES, 8, 16, NCH_DG).transpose(0, 2, 3, 1).reshape(NCORES, 16, NCH_DG * 8)
    dgidx_w = np.tile(lw, (1, 8, 1)).astype(np.int16)  # [NCORES, 128, NCH_DG*8]
    if PRE:
        # re-place each gather group's idx block at a 128B-aligned column
        gmeta = _dg_groups(C_th)
        NCHI = gmeta[-1][4] + gmeta[-1][3] if gmeta else 0
        NCHI = (NCHI + 7) // 8 * 8
        ali = np.zeros((NCORES, 128, NCHI * 8), np.int16)
        for (_, _, gs0, gw, gi0) in gmeta:
            ali[:, :, gi0 * 8:(gi0 + gw) * 8] = dgidx_w[:, :, gs0 * 8:(gs0 + gw) * 8]
        dgidx_w = ali

    # dis tiles [NCORES, P, NT]
    dis_t = np.zeros((NCORES, P, NT), np.float32)
    ids = np.arange(N)
    dis_t[ids // NSH, (ids % NSH) % P, (ids % NSH) // P] = dis

    # weights: [K, Dout] -> [P, (K//P)*Dout] with slice k at [:, k*Dout:(k+1)*Dout]
    def wfold(W, dt):
        K, Do = W.shape
        return np.ascontiguousarray(
            W.reshape(K // P, P, Do).transpose(1, 0, 2).reshape(P, (K // P) * Do)
        ).astype(dt)

    w1_sb = wfold(W1, BF16)
    w2_sb = wfold(W2, BF16)
    w3_sb = wfold(W3, BF16)
    wall_sb = wfold(np.concatenate([We, Wh, Wg], axis=1), BF16)  # [256,72]

    def brep(b):
        return np.broadcast_to(np.asarray(b, np.float32), (P, len(b))).copy()

    b1_rep = brep(b1)
    b2_rep = brep(b2)
    b3_rep = brep(b3)
    ball_rep = brep(np.concatenate([be, bh, bg]))

    # xT per core: [P, 4*NSHD], slice k = xpad_c[:, k*128:(k+1)*128].T
    xt = np.zeros((NCORES, P, 4 * NSHD), BF16)
    for c in range(NCORES):
        xc = np.zeros((NSHD, D_IN), np.float32)
        xc[:NSH] = x[c * NSH:(c + 1) * NSH]
        xt[c] = (
            xc.T.reshape(4, P, NSHD).transpose(1, 0, 2).reshape(P, 4 * NSHD)
        ).astype(BF16)

    NIOTA = max(SB, DGW, 1)
    iota_rep = np.broadcast_to(
        np.tile(np.arange(P, dtype=np.float32), NIOTA)[None, :],
        (P, NIOTA * P),
    ).copy()

    shared = dict(
        w1=w1_sb, w2=w2_sb, w3=w3_sb, wall=wall_sb,
        b1=b1_rep, b2=b2_rep, b3=b3_rep, ball=ball_rep, iota=iota_rep,
    )
    per_core = [
        dict(xt=xt[c], gidx=gidx[c], dstl=dstl[c], dis=dis_t[c],
             dgidx=dgidx_w[c], dstldg=dstl_dg[c])
        for c in range(NCORES)
    ]
    if DG:
        return shared, per_core, C_th, NCH_DG
    return shared, per_core, C_t, NCH


def _build(C_t, NCH, sim=False):
    import concourse.bass as bass
    import concourse.tile as tile
    from concourse import bacc, mybir
    from concourse.masks import make_identity

    f32 = mybir.dt.float32
    bf16 = mybir.dt.bfloat16
    i32 = mybir.dt.int32

    nc = bacc.Bacc("TRN2", target_bir_lowering=False, debug=False, num_devices=NCORES,
                   dynamic_dma_scratch_size=DMASCR, num_swdge_queues=NQ)
    NCHI = 0
    if DG:
        _g = _dg_groups(C_t)
        NCHI = (_g[-1][4] + _g[-1][3] + 7) // 8 * 8

    # I/O
    xt_in = nc.dram_tensor("xt", [P, 4 * NSHD], bf16, kind="ExternalInput")
    w1_in = nc.dram_tensor("w1", [P, 4 * D_H], bf16, kind="ExternalInput")
    w2_in = nc.dram_tensor("w2", [P, 4 * D_H], bf16, kind="ExternalInput")
    w3_in = nc.dram_tensor("w3", [P, 4 * D_E], bf16, kind="ExternalInput")
    wall_in = nc.dram_tensor("wall", [P, 2 * 72], bf16, kind="ExternalInput")
    b1_in = nc.dram_tensor("b1", [P, D_H], f32, kind="ExternalInput")
    b2_in = nc.dram_tensor("b2", [P, D_H], f32, kind="ExternalInput")
    b3_in = nc.dram_tensor("b3", [P, D_E], f32, kind="ExternalInput")
    ball_in = nc.dram_tensor("ball", [P, 72], f32, kind="ExternalInput")
    i16 = mybir.dt.int16
    if DG:
        dgidx_in = nc.dram_tensor("dgidx", [P, (NCHI if PRE else NCH) * 8], i16, kind="ExternalInput")
        dstl_in = nc.dram_tensor("dstldg", [P, NCH], f32, kind="ExternalInput")
    else:
        gidx_in = nc.dram_tensor("gidx", [P, NCH], i32, kind="ExternalInput")
        dstl_in = nc.dram_tensor("dstl", [P, NCH], f32, kind="ExternalInput")
    dis_in = nc.dram_tensor("dis", [P, NT], f32, kind="ExternalInput")
    NIOTA = max(SB, DGW, 1)
    iota_in = nc.dram_tensor("iota", [P, NIOTA * P], f32, kind="ExternalInput")
    emb_out = nc.dram_tensor("emb", [NSH, D_E], f32, kind="ExternalOutput")
    heads_out = nc.dram_tensor("heads", [NSH, 72], f32, kind="ExternalOutput")

    # per-column chunk metadata (host-known, compile-time)
    if DG:
        Ct_tot = np.asarray(C_t).sum(axis=1)           # [NT] total chunks/tile
        groups = _dg_groups(C_t)
        assert groups[-1][2] + groups[-1][3] == NCH
        NCHI = (groups[-1][4] + groups[-1][3] + 7) // 8 * 8
    else:
        Ct_tot = np.asarray(C_t)
    col_tile = np.repeat(np.arange(NT), Ct_tot)        # tile id per column
    col_first = np.zeros(NCH, bool)
    col_last = np.zeros(NCH, bool)
    cb = np.concatenate([[0], np.cumsum(Ct_tot)])
    col_first[cb[:-1]] = True
    col_last[cb[1:] - 1] = True

    rg = [list(range(NCORES))]

    with tile.TileContext(nc) as tc:
        with (
            tc.tile_pool(name="const", bufs=1) as const,
            tc.tile_pool(name="mpool", bufs=3) as mpool,
            tc.tile_pool(name="gpool", bufs=4) as gpool,
            tc.tile_pool(name="spool", bufs=4) as spool,
            tc.tile_pool(name="hpool", bufs=3) as hpool,
            tc.tile_pool(name="ipool", bufs=3) as ipool,
            tc.tile_pool(name="psg", bufs=2, space="PSUM") as psg,
            tc.tile_pool(name="psa", bufs=int(os.environ.get("GCN_PSA","3")), space="PSUM") as psa,
            tc.tile_pool(name="pst", bufs=2, space="PSUM") as pst,
            tc.tile_pool(name="dram", bufs=1, space="DRAM") as dram,
        ):
            # ---- persistent tables ----
            ht = const.tile([P, 4 * NSHD], bf16, tag="ht")
            nc.sync.dma_start(ht[:], xt_in[:])
            embT = const.tile([P, 2 * NSHD], bf16, tag="embT")
            w1 = const.tile([P, 4 * D_H], bf16, tag="w1")
            nc.sync.dma_start(w1[:], w1_in[:])
            w2 = const.tile([P, 4 * D_H], bf16, tag="w2")
            nc.sync.dma_start(w2[:], w2_in[:])
            w3 = const.tile([P, 4 * D_E], bf16, tag="w3")
            nc.sync.dma_start(w3[:], w3_in[:])
            wall = const.tile([P, 2 * 72], bf16, tag="wall")
            nc.sync.dma_start(wall[:], wall_in[:])
            b1r = const.tile([P, D_H], f32, tag="b1")
            nc.sync.dma_start(b1r[:], b1_in[:])
            b2r = const.tile([P, D_H], f32, tag="b2")
            nc.sync.dma_start(b2r[:], b2_in[:])
            b3r = const.tile([P, D_E], f32, tag="b3")
            nc.sync.dma_start(b3r[:], b3_in[:])
            ballr = const.tile([P, 72], f32, tag="ball")
            nc.sync.dma_start(ballr[:], ball_in[:])
            if DG and PRE:
                dgidx_sb = const.tile([P, NCHI * 8], i16, tag="dgidx")
                nc.sync.dma_start(dgidx_sb[:], dgidx_in[:])
            if not DG:
                gidx_sb = const.tile([P, NCH], i32, tag="gidx")
                nc.sync.dma_start(gidx_sb[:], gidx_in[:])
            dstl_sb = const.tile([P, NCH], f32, tag="dstl")
            nc.sync.dma_start(dstl_sb[:], dstl_in[:])
            dis_sb = const.tile([P, NT], f32, tag="dis")
            nc.sync.dma_start(dis_sb[:], dis_in[:])
            iota_sb = const.tile([P, NIOTA * P], f32, tag="iota")
            nc.sync.dma_start(iota_sb[:], iota_in[:])
            ident_bf = const.tile([P, P], bf16, tag="identb")
            make_identity(nc, ident_bf[:])
            ident_f32 = const.tile([P, P], f32, tag="identf")
            make_identity(nc, ident_f32[:])

            # DRAM staging
            m1 = dram.tile([NSHD, D_H], bf16, tag="m1")
            ag1 = dram.tile([NCORES * NSHD, D_H], bf16, tag="ag1")
            m2 = dram.tile([NSHD, D_H], bf16, tag="m2")
            ag2 = dram.tile([NCORES * NSHD, D_H], bf16, tag="ag2")
            m3 = dram.tile([NSHD, D_E], bf16, tag="m3")
            ag3 = dram.tile([NCORES * NSHD, D_E], bf16, tag="ag3")

            convs = [
                (D_H, w1, b1r, m1, ag1, True),
                (D_H, w2, b2r, m2, ag2, True),
                (D_E, w3, b3r, m3, ag3, False),
            ][:NCONVS]
            TL = NT_LIM or NT
            NCH_L = int(cb[TL])

            for li, (Do, w_sb, b_rep, mview, agview, relu) in enumerate(convs):
                # ---- GEMM: M' = dis * (H @ W) ----
                with nc.named_scope(f"gemm{li}"):
                    for t in range(TL):
                        ps = psg.tile([P, D_H], mybir.dt.float32, space="PSUM", tag="psg")
                        for k in range(4):
                            nc.tensor.matmul(
                                ps[:, :Do],
                                lhsT=ht[:, k * NSHD + t * P: k * NSHD + (t + 1) * P],
                                rhs=w_sb[:, k * Do:(k + 1) * Do],
                                start=(k == 0),
                                stop=(k == 3),
                            )
                        mt = mpool.tile([P, D_H], bf16, tag="mt")
                        nc.scalar.mul(mt[:, :Do], ps[:, :Do], dis_sb[:, t:t + 1])
                        nc.sync.dma_start(mview[t * P:(t + 1) * P, :], mt[:, :Do])

                # ---- AllGather M' ----
                with nc.named_scope(f"ag{li}"):
                    if sim:
                        # single-core cost-model stand-in (no collectives in sim)
                        nc.gpsimd.dma_start(agview[0:NSHD, :], mview[:])
                    else:
                        nc.gpsimd.collective_compute(
                            "AllGather",
                            mybir.AluOpType.bypass,
                            replica_groups=rg,
                            ins=[mview[:].opt()],
                            outs=[agview[:].opt()],
                        )

                def _evac(t, nt, ps, Do=Do, b_rep=b_rep, relu=relu):
                    t1 = hpool.tile([P, D_H], mybir.dt.float32, tag="h32")
                    nc.vector.tensor_scalar(
                        out=t1[:, :Do], in0=ps[:, :Do],
                        scalar1=dis_sb[:, t:t + 1], scalar2=None,
                        op0=mybir.AluOpType.mult,
                    )
                    nc.vector.tensor_tensor(
                        out=t1[:, :Do], in0=t1[:, :Do],
                        in1=b_rep[:, :Do], op=mybir.AluOpType.add,
                    )
                    if relu:
                        hb = hpool.tile([P, D_H], bf16, tag="hb")
                        nc.scalar.activation(
                            hb[:, :Do], t1[:, :Do],
                            mybir.ActivationFunctionType.Relu,
                        )
                        for k in range(4):
                            tp = pst.tile([P, P], bf16, space="PSUM", tag="tp")
                            nc.tensor.transpose(
                                tp[:], hb[:, k * P:(k + 1) * P], ident_bf[:]
                            )
                            nc.vector.tensor_copy(
                                ht[:, k * NSHD + t * P: k * NSHD + (t + 1) * P],
                                tp[:],
                            )
                    else:
                        nc.sync.dma_start(
                            emb_out[t * P:t * P + nt, :], t1[:nt, :Do]
                        )
                        for k in range(2):
                            tp = pst.tile([P, P], mybir.dt.float32, space="PSUM", tag="tp")
                            nc.tensor.transpose(
                                tp[:], t1[:, k * P:(k + 1) * P], ident_f32[:]
                            )
                            nc.vector.tensor_copy(
                                embT[:, k * NSHD + t * P: k * NSHD + (t + 1) * P],
                                tp[:],
                            )

                # ---- Aggregation ----
                with nc.named_scope(f"agg{li}"):
                    ps = None
                    if DG:
                        for gno, (t, h, gc0, Wg, ic0) in enumerate(groups):
                            if t >= TL:
                                continue
                            if PRE:
                                iap = dgidx_sb[:, ic0 * 8:(ic0 + Wg) * 8]
                            else:
                                gi = ipool.tile([P, DGW * 8], i16, tag="gi")
                                nc.sync.dma_start(gi[:, :Wg * 8], dgidx_in[:, gc0 * 8:(gc0 + Wg) * 8])
                                iap = gi[:, :Wg * 8]
                            gt_ = gpool.tile([P, DGW * D_H], bf16, tag="g")
                            nc.gpsimd.dma_gather(
                                out_ap=gt_[:, :Wg * Do].rearrange("p (a b) -> p a b", b=Do),
                                in_ap=agview[h * HALF:(h + 1) * HALF, :],
                                idxs_ap=iap,
                                num_idxs=Wg * P,
                                num_idxs_reg=Wg * P,
                                elem_size=Do,
                                queue_num=gno % NQ,
                            )
                            st_ = spool.tile([P, DGW * P], bf16, tag="s")
                            nc.vector.tensor_tensor(
                                out=st_[:, :Wg * P].rearrange("p (a b) -> p a b", a=Wg),
                                in0=dstl_sb[:, gc0:gc0 + Wg].to_broadcast([P, Wg, P]),
                                in1=iota_sb[:, :Wg * P].rearrange("p (a b) -> p a b", a=Wg),
                                op=mybir.AluOpType.is_equal,
                            )
                            for j in range(Wg):
                                c = gc0 + j
                                nt = min(P, NSH - t * P)
                                if col_first[c]:
                                    ps = psa.tile([P, D_H], mybir.dt.float32, space="PSUM", tag="psa")
                                nc.tensor.matmul(
                                    ps[:, :Do],
                                    lhsT=st_[:, j * P:(j + 1) * P],
                                    rhs=gt_[:, j * Do:(j + 1) * Do],
                                    start=bool(col_first[c]),
                                    stop=bool(col_last[c]),
                                )
                                if col_last[c]:
                                    _evac(t, nt, ps)
                    BN = max(GB, SB)
                    for b0 in ([] if DG else range(0, NCH_L, BN)):
                        nb = min(BN, NCH_L - b0)
                        # gather batch
                        gt_ = gpool.tile([P, BN * D_H], bf16, tag="g")
                        for j0 in range(0, nb, GB):
                            jn = min(GB, nb - j0)
                            nc.gpsimd.indirect_dma_start(
                                out=gt_[:, j0 * Do:(j0 + jn) * Do],
                                out_offset=None,
                                in_=agview[:],
                                in_offset=bass.IndirectOffsetOnAxis(
                                    ap=gidx_sb[:, b0 + j0:b0 + j0 + jn], axis=0
                                ),
                            )
                        # S batch
                        st_ = spool.tile([P, BN * P], bf16, tag="s")
                        for j0 in range(0, nb, SB):
                            jn = min(SB, nb - j0)
                            nc.vector.tensor_tensor(
                                out=st_[:, j0 * P:(j0 + jn) * P].rearrange(
                                    "p (a b) -> p a b", a=jn
                                ),
                                in0=dstl_sb[:, b0 + j0:b0 + j0 + jn].to_broadcast(
                                    [P, jn, P]
                                ),
                                in1=iota_sb[:, :jn * P].rearrange("p (a b) -> p a b", a=jn),
                                op=mybir.AluOpType.is_equal,
                            )
                        for j in range(nb):
                            c = b0 + j
                            t = int(col_tile[c])
                            nt = min(P, NSH - t * P)
                            if col_first[c]:
                                ps = psa.tile([P, D_H], mybir.dt.float32, space="PSUM", tag="psa")
                            nc.tensor.matmul(
                                ps[:, :Do],
                                lhsT=st_[:, j * P:(j + 1) * P],
                                rhs=gt_[:, j * Do:(j + 1) * Do],
                                start=bool(col_first[c]),
                                stop=bool(col_last[c]),
                            )
                            if col_last[c]:
                                _evac(t, nt, ps)

            # ---- heads ----
            with nc.named_scope("heads"):
                for t in range(TL if NCONVS >= 3 else 0):
                    nt = min(P, NSH - t * P)
                    ps = pst.tile([P, 72], mybir.dt.float32, space="PSUM", tag="tp")
                    for k in range(2):
                        nc.tensor.matmul(
                            ps[:, :],
                            lhsT=embT[:, k * NSHD + t * P: k * NSHD + (t + 1) * P],
                            rhs=wall[:, k * 72:(k + 1) * 72],
                            start=(k == 0),
                            stop=(k == 1),
                        )
                    hh = hpool.tile([P, 72], mybir.dt.float32, tag="hh")
                    nc.vector.tensor_tensor(
                        out=hh[:, :], in0=ps[:, :], in1=ballr[:, :],
                        op=mybir.AluOpType.add,
                    )
                    nc.sync.dma_start(heads_out[t * P:t * P + nt, :], hh[:nt, :])

    nc.compile()
    return nc


_CACHE = {}


def _run(inputs, trace=False):
    from concourse.bass_utils import run_bass_kernel_spmd

    x = np.asarray(inputs["x"], np.float32)
    edge_index = np.asarray(inputs["edge_index"])
    args = {k: np.asarray(inputs[k], np.float32) for k in
            ["W1", "b1", "W2", "b2", "W3", "b3", "We", "be", "Wh", "bh", "Wg", "bg"]}

    shared, per_core, C_t, NCH = _prep(x, edge_index, **args)

    key = ("prog", NCH, tuple(np.asarray(C_t).reshape(-1).tolist()))
    if key not in _CACHE:
        _CACHE[key] = _build(C_t, NCH)
    nc = _CACHE[key]

    in_maps = [{**shared, **pc} for pc in per_core]
    global _LAST
    _LAST = (nc, in_maps)
    res = run_bass_kernel_spmd(nc, in_maps, core_ids=list(range(NCORES)), trace=trace)

    emb = np.concatenate([res.results[c]["emb"] for c in range(NCORES)], axis=0)
    heads = np.concatenate([res.results[c]["heads"] for c in range(NCORES)], axis=0)
    out = (emb, heads[:, :7], heads[:, 7:15], heads[:, 15:])
    return out, res


def kernel(**inputs):
    out, _ = _run(inputs, trace=False)
    return out


# revision 37
# speedup vs baseline: 35.6662x; 35.6662x over previous
"""3-layer GCN (AccessibilityGNN) on 8 Trainium2 NeuronCores.

Strategy (graph/data parallel, per sharding hint):
- Nodes row-sharded 6250/core. Per conv: local GEMM M = H @ W (bf16, PSUM fp32),
  fold dis[src] into M via per-partition ACT scale, AllGather M' (bf16) across
  the 8 cores, then each core aggregates its own dst rows: per 128-dst tile,
  indirect-DMA gather of source rows + one-hot (0/1) matmul accumulation in
  PSUM (S built on device via is_equal against an iota tile). Evacuate with
  dis[dst] scale + bias (+ relu for convs 1-2). H for the next conv's GEMM is
  produced feature-major via PE transposes. Heads = one fused [256,72] GEMM.

Host side: degree/normalization, edge sorting/chunking/padding, input
transpose+casts, output unshard.
"""
import os
import sys

sys.path.insert(0, "/opt/trn_rl_repo")

import numpy as np
import ml_dtypes

N = 50000
E = 800000
D_IN = 512
D_H = 512
D_E = 256
NCORES = 8
NSH = N // NCORES  # 6250
P = 128
NT = (NSH + P - 1) // P  # 49 tiles; last tile has 106 rows
NSHD = NT * P  # device-padded shard rows (6272) so all compute tiles are full

BF16 = ml_dtypes.bfloat16

# toggles (env-overridable for experiments)
# NOTE: multi-index indirect DMA (GB>1) silently gathers wrong rows on HW —
# only the first index column is honored. Keep GB=1.
GB = int(os.environ.get("GCN_GATHER_BATCH", "1"))   # chunks per indirect DMA
SB = int(os.environ.get("GCN_ISEQ_BATCH", "8"))     # chunks per is_equal
TRACE = bool(int(os.environ.get("GCN_TRACE", "0")))
NT_LIM = int(os.environ.get("GCN_NT_LIM", "0")) or None   # debug: limit tiles/conv
NCONVS = int(os.environ.get("GCN_CONVS", "3"))            # debug: number of convs
DG = bool(int(os.environ.get("GCN_DG", "1")))             # use dma_gather batching
DGW = int(os.environ.get("GCN_DGW", "7"))                # chunks per dma_gather
DMASCR = int(os.environ.get("GCN_DMASCR", "16384"))        # SWDGE ring bytes (1024 descs default)
NQ = int(os.environ.get("GCN_NQ", "1"))                    # SWDGE queues for gathers
PRE = bool(int(os.environ.get("GCN_PRE", "1")))            # preload idx table (128B-aligned slices)
SAG = bool(int(os.environ.get("GCN_SAG", "0")))            # staged AllGather (2 halves, overlap)
HSH = 3136                                                 # half-shard rows (NSHD/2)
HALF = NCORES * NSHD // 2                                  # gather-table half rows


def _dg_groups(C_th):
    """Group split: (tile, half, S-col start, width, aligned idx-col start)."""
    out = []
    gc = 0
    ic = 0
    for t in range(NT):
        for h in range(2):
            c_rem = int(C_th[t][h])
            while c_rem > 0:
                w = min(DGW, c_rem)
                ic = (ic + 7) // 8 * 8  # 8 chunks * 16B = 128B alignment
                out.append((t, h, gc, w, ic))
                gc += w
                ic += w
                c_rem -= w
    return out


def _prep(x, edge_index, W1, b1, W2, b2, W3, b3, We, be, Wh, bh, Wg, bg):
    """Host-side sharding + edge-plan construction."""
    src = edge_index[0].astype(np.int64)
    dst = edge_index[1].astype(np.int64)
    loops = np.arange(N, dtype=np.int64)
    src_all = np.concatenate([src, loops])
    dst_all = np.concatenate([dst, loops])

    deg = np.bincount(dst_all, minlength=N).astype(np.float64)
    dis = (1.0 / np.sqrt(deg)).astype(np.float32)

    order = np.argsort(dst_all, kind="stable")
    s_sorted = src_all[order]
    d_sorted = dst_all[order]

    core_of = d_sorted // NSH
    t_of = (d_sorted % NSH) // P
    gt = core_of * NT + t_of  # global (core,tile) id; monotone since d_sorted sorted
    counts = np.bincount(gt, minlength=NCORES * NT)
    starts = np.concatenate([[0], np.cumsum(counts)])

    # common per-tile chunk count across cores
    cmat = counts.reshape(NCORES, NT)
    C_t = np.maximum(1, (cmat.max(axis=0) + P - 1) // P).astype(np.int64)  # [NT]
    col_base = np.concatenate([[0], np.cumsum(C_t)])
    NCH = int(col_base[-1])

    gidx = np.zeros((NCORES, P, NCH), np.int32)
    dstl = np.full((NCORES, P, NCH), -1.0, np.float32)
    r = np.arange(len(d_sorted)) - starts[gt]
    col = col_base[t_of] + r // P
    row = r % P
    # remap to device-padded row ids: node (c, j) lives at row c*NSHD + j
    r_glob = ((s_sorted // NSH) * NSHD + (s_sorted % NSH)).astype(np.int64)
    gidx[core_of, row, col] = r_glob.astype(np.int32)
    dstl[core_of, row, col] = ((d_sorted % NSH) - t_of * P).astype(np.float32)

    # ---- dma_gather plan: edges regrouped by (core, tile, table-half) ----
    if SAG:
        # half = position within the owning core's shard; table half h is then
        # exactly the output of one AllGather over mview[h*HSH:(h+1)*HSH]
        h_of = ((r_glob % NSHD) // HSH).astype(np.int64)
    else:
        h_of = (r_glob // HALF).astype(np.int64)
    grp = (core_of * NT + t_of) * 2 + h_of
    order2 = np.argsort(grp, kind="stable")
    g2 = grp[order2]
    cnts = np.bincount(g2, minlength=NCORES * NT * 2)
    C_th = np.ceil(cnts.reshape(NCORES, NT, 2).max(axis=0) / P).astype(np.int64)  # [NT,2]
    colbase_th = np.concatenate([[0], np.cumsum(C_th.reshape(-1))]).reshape(-1)
    NCH_DG = int(colbase_th[-1])
    starts2 = np.concatenate([[0], np.cumsum(cnts)])
    rank2 = np.arange(len(g2)) - starts2[g2]
    t2 = t_of[order2]
    h2 = h_of[order2]
    col2 = colbase_th[t2 * 2 + h2] + rank2 // P
    row2 = rank2 % P
    lidx = np.zeros((NCORES, P, NCH_DG), np.int64)   # local (half) row ids, pad=0
    dstl_dg = np.full((NCORES, P, NCH_DG), -1.0, np.float32)
    if SAG:
        lidx[core_of[order2], row2, col2] = (
            (r_glob[order2] // NSHD) * HSH + (r_glob[order2] % NSHD) % HSH
        )
    else:
        lidx[core_of[order2], row2, col2] = r_glob[order2] % HALF
    dstl_dg[core_of[order2], row2, col2] = ((d_sorted[order2] % NSH) - t2 * P).astype(np.float32)
    # wrap per chunk: dgidx_w[16*rep + q, col*8 + c2] = lidx[c2*16 + q, col]
    lw = lidx.reshape(NCORES, 8, 16, NCH_DG).transpose(0, 2, 3, 1).reshape(NCORES, 16, NCH_DG * 8)
    dgidx_w = np.tile(lw, (1, 8, 1)).astype(np.int16)  # [NCORES, 128, NCH_DG*8]
    if PRE:
        # re-place each gather group's idx block at a 128B-aligned column
        gmeta = _dg_groups(C_th)
        NCHI = gmeta[-1][4] + gmeta[-1][3] if gmeta else 0
        NCHI = (NCHI + 7) // 8 * 8
        ali = np.zeros((NCORES, 128, NCHI * 8), np.int16)
        for (_, _, gs0, gw, gi0) in gmeta:
            ali[:, :, gi0 * 8:(gi0 + gw) * 8] = dgidx_w[:, :, gs0 * 8:(gs0 + gw) * 8]
        dgidx_w = ali

    # dis tiles [NCORES, P, NT]
    dis_t = np.zeros((NCORES, P, NT), np.float32)
    ids = np.arange(N)
    dis_t[ids // NSH, (ids % NSH) % P, (ids % NSH) // P] = dis

    # weights: [K, Dout] -> [P, (K//P)*Dout] with slice k at [:, k*Dout:(k+1)*Dout]
    def wfold(W, dt):
        K, Do = W.shape
        return np.ascontiguousarray(
            W.reshape(K // P, P, Do).transpose(1, 0, 2).reshape(P, (K // P) * Do)
        ).astype(dt)

    w1_sb = wfold(W1, BF16)
    w2_sb = wfold(W2, BF16)
    w3_sb = wfold(W3, BF16)
    wall_sb = wfold(np.concatenate([We, Wh, Wg], axis=1), BF16)  # [256,72]

    def brep(b):
        return np.broadcast_to(np.asarray(b, np.float32), (P, len(b))).copy()

    b1_rep = brep(b1)
    b2_rep = brep(b2)
    b3_rep = brep(b3)
    ball_rep = brep(np.concatenate([be, bh, bg]))

    # xT per core: [P, 4*NSHD], slice k = xpad_c[:, k*128:(k+1)*128].T
    xt = np.zeros((NCORES, P, 4 * NSHD), BF16)
    for c in range(NCORES):
        xc = np.zeros((NSHD, D_IN), np.float32)
        xc[:NSH] = x[c * NSH:(c + 1) * NSH]
        xt[c] = (
            xc.T.reshape(4, P, NSHD).transpose(1, 0, 2).reshape(P, 4 * NSHD)
        ).astype(BF16)

    NIOTA = max(SB, DGW, 1)
    iota_rep = np.broadcast_to(
        np.tile(np.arange(P, dtype=np.float32), NIOTA)[None, :],
        (P, NIOTA * P),
    ).copy()

    shared = dict(
        w1=w1_sb, w2=w2_sb, w3=w3_sb, wall=wall_sb,
        b1=b1_rep, b2=b2_rep, b3=b3_rep, ball=ball_rep, iota=iota_rep,
    )
    per_core = [
        dict(xt=xt[c], gidx=gidx[c], dstl=dstl[c], dis=dis_t[c],
             dgidx=dgidx_w[c], dstldg=dstl_dg[c])
        for c in range(NCORES)
    ]
    if DG:
        return shared, per_core, C_th, NCH_DG
    return shared, per_core, C_t, NCH


def _build(C_t, NCH, sim=False):
    import concourse.bass as bass
    import concourse.tile as tile
    from concourse import bacc, mybir
    from concourse.masks import make_identity

    f32 = mybir.dt.float32
    bf16 = mybir.dt.bfloat16
    i32 = mybir.dt.int32

    nc = bacc.Bacc("TRN2", target_bir_lowering=False, debug=False, num_devices=NCORES,
                   dynamic_dma_scratch_size=DMASCR, num_swdge_queues=NQ)
    NCHI = 0
    if DG:
        _g = _dg_groups(C_t)
        NCHI = (_g[-1][4] + _g[-1][3] + 7) // 8 * 8

    # I/O
    xt_in = nc.dram_tensor("xt", [P, 4 * NSHD], bf16, kind="ExternalInput")
    w1_in = nc.dram_tensor("w1", [P, 4 * D_H], bf16, kind="ExternalInput")
    w2_in = nc.dram_tensor("w2", [P, 4 * D_H], bf16, kind="ExternalInput")
    w3_in = nc.dram_tensor("w3", [P, 4 * D_E], bf16, kind="ExternalInput")
    wall_in = nc.dram_tensor("wall", [P, 2 * 72], bf16, kind="ExternalInput")
    b1_in = nc.dram_tensor("b1", [P, D_H], f32, kind="ExternalInput")
    b2_in = nc.dram_tensor("b2", [P, D_H], f32, kind="ExternalInput")
    b3_in = nc.dram_tensor("b3", [P, D_E], f32, kind="ExternalInput")
    ball_in = nc.dram_tensor("ball", [P, 72], f32, kind="ExternalInput")
    i16 = mybir.dt.int16
    if DG:
        dgidx_in = nc.dram_tensor("dgidx", [P, (NCHI if PRE else NCH) * 8], i16, kind="ExternalInput")
        dstl_in = nc.dram_tensor("dstldg", [P, NCH], f32, kind="ExternalInput")
    else:
        gidx_in = nc.dram_tensor("gidx", [P, NCH], i32, kind="ExternalInput")
        dstl_in = nc.dram_tensor("dstl", [P, NCH], f32, kind="ExternalInput")
    dis_in = nc.dram_tensor("dis", [P, NT], f32, kind="ExternalInput")
    NIOTA = max(SB, DGW, 1)
    iota_in = nc.dram_tensor("iota", [P, NIOTA * P], f32, kind="ExternalInput")
    emb_out = nc.dram_tensor("emb", [NSH, D_E], f32, kind="ExternalOutput")
    heads_out = nc.dram_tensor("heads", [NSH, 72], f32, kind="ExternalOutput")

    # per-column chunk metadata (host-known, compile-time)
    if DG:
        Ct_tot = np.asarray(C_t).sum(axis=1)           # [NT] total chunks/tile
        groups = _dg_groups(C_t)
        assert groups[-1][2] + groups[-1][3] == NCH
        NCHI = (groups[-1][4] + groups[-1][3] + 7) // 8 * 8
    else:
        Ct_tot = np.asarray(C_t)
    col_tile = np.repeat(np.arange(NT), Ct_tot)        # tile id per column
    col_first = np.zeros(NCH, bool)
    col_last = np.zeros(NCH, bool)
    cb = np.concatenate([[0], np.cumsum(Ct_tot)])
    col_first[cb[:-1]] = True
    col_last[cb[1:] - 1] = True

    rg = [list(range(NCORES))]

    with tile.TileContext(nc) as tc:
        with (
            tc.tile_pool(name="const", bufs=1) as const,
            tc.tile_pool(name="mpool", bufs=3) as mpool,
            tc.tile_pool(name="gpool", bufs=4) as gpool,
            tc.tile_pool(name="spool", bufs=4) as spool,
            tc.tile_pool(name="hpool", bufs=3) as hpool,
            tc.tile_pool(name="ipool", bufs=3) as ipool,
            tc.tile_pool(name="psg", bufs=2, space="PSUM") as psg,
            tc.tile_pool(name="psa", bufs=int(os.environ.get("GCN_PSA","3")), space="PSUM") as psa,
            tc.tile_pool(name="pst", bufs=2, space="PSUM") as pst,
            tc.tile_pool(name="dram", bufs=1, space="DRAM") as dram,
        ):
            # ---- persistent tables ----
            ht = const.tile([P, 4 * NSHD], bf16, tag="ht")
            nc.sync.dma_start(ht[:], xt_in[:])
            embT = const.tile([P, 2 * NSHD], bf16, tag="embT")
            w1 = const.tile([P, 4 * D_H], bf16, tag="w1")
            nc.sync.dma_start(w1[:], w1_in[:])
            w2 = const.tile([P, 4 * D_H], bf16, tag="w2")
            nc.sync.dma_start(w2[:], w2_in[:])
            w3 = const.tile([P, 4 * D_E], bf16, tag="w3")
            nc.sync.dma_start(w3[:], w3_in[:])
            wall = const.tile([P, 2 * 72], bf16, tag="wall")
            nc.sync.dma_start(wall[:], wall_in[:])
            b1r = const.tile([P, D_H], f32, tag="b1")
            nc.sync.dma_start(b1r[:], b1_in[:])
            b2r = const.tile([P, D_H], f32, tag="b2")
            nc.sync.dma_start(b2r[:], b2_in[:])
            b3r = const.tile([P, D_E], f32, tag="b3")
            nc.sync.dma_start(b3r[:], b3_in[:])
            ballr = const.tile([P, 72], f32, tag="ball")
            nc.sync.dma_start(ballr[:], ball_in[:])
            if DG and PRE:
                dgidx_sb = const.tile([P, NCHI * 8], i16, tag="dgidx")
                nc.sync.dma_start(dgidx_sb[:], dgidx_in[:])
            if not DG:
                gidx_sb = const.tile([P, NCH], i32, tag="gidx")
                nc.sync.dma_start(gidx_sb[:], gidx_in[:])
            dstl_sb = const.tile([P, NCH], f32, tag="dstl")
            nc.sync.dma_start(dstl_sb[:], dstl_in[:])
            dis_sb = const.tile([P, NT], f32, tag="dis")
            nc.sync.dma_start(dis_sb[:], dis_in[:])
            iota_sb = const.tile([P, NIOTA * P], f32, tag="iota")
            nc.sync.dma_start(iota_sb[:], iota_in[:])
            ident_bf = const.tile([P, P], bf16, tag="identb")
            make_identity(nc, ident_bf[:])
            ident_f32 = const.tile([P, P], f32, tag="identf")
            make_identity(nc, ident_f32[:])

            # DRAM staging; in SAG mode each "agX" is a pair of half tables
            m1 = dram.tile([NSHD, D_H], bf16, tag="m1")
            m2 = dram.tile([NSHD, D_H], bf16, tag="m2")
            m3 = dram.tile([NSHD, D_E], bf16, tag="m3")
            if SAG:
                ag1a = dram.tile([HALF, D_H], bf16, tag="ag1a")
                ag1b = dram.tile([HALF, D_H], bf16, tag="ag1b")
                ag2a = dram.tile([HALF, D_H], bf16, tag="ag2a")
                ag2b = dram.tile([HALF, D_H], bf16, tag="ag2b")
                ag3a = dram.tile([HALF, D_E], bf16, tag="ag3a")
                ag3b = dram.tile([HALF, D_E], bf16, tag="ag3b")
                ag1, ag2, ag3 = (ag1a, ag1b), (ag2a, ag2b), (ag3a, ag3b)
            else:
                ag1 = dram.tile([NCORES * NSHD, D_H], bf16, tag="ag1")
                ag2 = dram.tile([NCORES * NSHD, D_H], bf16, tag="ag2")
                ag3 = dram.tile([NCORES * NSHD, D_E], bf16, tag="ag3")

            convs = [
                (D_H, w1, b1r, m1, ag1, True),
                (D_H, w2, b2r, m2, ag2, True),
                (D_E, w3, b3r, m3, ag3, False),
            ][:NCONVS]
            TL = NT_LIM or NT
            NCH_L = int(cb[TL])

            for li, (Do, w_sb, b_rep, mview, agview, relu) in enumerate(convs):
                # ---- GEMM: M' = dis * (H @ W) ----
                with nc.named_scope(f"gemm{li}"):
                    for t in range(TL):
                        ps = psg.tile([P, D_H], mybir.dt.float32, space="PSUM", tag="psg")
                        for k in range(4):
                            nc.tensor.matmul(
                                ps[:, :Do],
                                lhsT=ht[:, k * NSHD + t * P: k * NSHD + (t + 1) * P],
                                rhs=w_sb[:, k * Do:(k + 1) * Do],
                                start=(k == 0),
                                stop=(k == 3),
                            )
                        mt = mpool.tile([P, D_H], bf16, tag="mt")
                        nc.scalar.mul(mt[:, :Do], ps[:, :Do], dis_sb[:, t:t + 1])
                        nc.sync.dma_start(mview[t * P:(t + 1) * P, :], mt[:, :Do])

                # ---- AllGather M' ----
                with nc.named_scope(f"ag{li}"):
                    if SAG:
                        for h in range(2):
                            if sim:
                                nc.gpsimd.dma_start(
                                    agview[h][0:HSH, :], mview[h * HSH:(h + 1) * HSH, :]
                                )
                            else:
                                nc.gpsimd.collective_compute(
                                    "AllGather",
                                    mybir.AluOpType.bypass,
                                    replica_groups=rg,
                                    ins=[mview[h * HSH:(h + 1) * HSH, :].opt()],
                                    outs=[agview[h][:].opt()],
                                )
                    elif sim:
                        # single-core cost-model stand-in (no collectives in sim)
                        nc.gpsimd.dma_start(agview[0:NSHD, :], mview[:])
                    else:
                        nc.gpsimd.collective_compute(
                            "AllGather",
                            mybir.AluOpType.bypass,
                            replica_groups=rg,
                            ins=[mview[:].opt()],
                            outs=[agview[:].opt()],
                        )

                def _evac(t, nt, ps, Do=Do, b_rep=b_rep, relu=relu):
                    t1 = hpool.tile([P, D_H], mybir.dt.float32, tag="h32")
                    nc.vector.tensor_scalar(
                        out=t1[:, :Do], in0=ps[:, :Do],
                        scalar1=dis_sb[:, t:t + 1], scalar2=None,
                        op0=mybir.AluOpType.mult,
                    )
                    nc.vector.tensor_tensor(
                        out=t1[:, :Do], in0=t1[:, :Do],
                        in1=b_rep[:, :Do], op=mybir.AluOpType.add,
                    )
                    if relu:
                        hb = hpool.tile([P, D_H], bf16, tag="hb")
                        nc.scalar.activation(
                            hb[:, :Do], t1[:, :Do],
                            mybir.ActivationFunctionType.Relu,
                        )
                        for k in range(4):
                            tp = pst.tile([P, P], bf16, space="PSUM", tag="tp")
                            nc.tensor.transpose(
                                tp[:], hb[:, k * P:(k + 1) * P], ident_bf[:]
                            )
                            nc.vector.tensor_copy(
                                ht[:, k * NSHD + t * P: k * NSHD + (t + 1) * P],
                                tp[:],
                            )
                    else:
                        nc.sync.dma_start(
                            emb_out[t * P:t * P + nt, :], t1[:nt, :Do]
                        )
                        for k in range(2):
                            tp = pst.tile([P, P], mybir.dt.float32, space="PSUM", tag="tp")
                            nc.tensor.transpose(
                                tp[:], t1[:, k * P:(k + 1) * P], ident_f32[:]
                            )
                            nc.vector.tensor_copy(
                                embT[:, k * NSHD + t * P: k * NSHD + (t + 1) * P],
                                tp[:],
                            )

                # ---- Aggregation ----
                with nc.named_scope(f"agg{li}"):
                    ps = None
                    if DG:
                        for gno, (t, h, gc0, Wg, ic0) in enumerate(groups):
                            if t >= TL:
                                continue
                            if PRE:
                                iap = dgidx_sb[:, ic0 * 8:(ic0 + Wg) * 8]
                            else:
                                gi = ipool.tile([P, DGW * 8], i16, tag="gi")
                                nc.sync.dma_start(gi[:, :Wg * 8], dgidx_in[:, gc0 * 8:(gc0 + Wg) * 8])
                                iap = gi[:, :Wg * 8]
                            gt_ = gpool.tile([P, DGW * D_H], bf16, tag="g")
                            nc.gpsimd.dma_gather(
                                out_ap=gt_[:, :Wg * Do].rearrange("p (a b) -> p a b", b=Do),
                                in_ap=(agview[h][:, :] if SAG
                                       else agview[h * HALF:(h + 1) * HALF, :]),
                                idxs_ap=iap,
                                num_idxs=Wg * P,
                                num_idxs_reg=Wg * P,
                                elem_size=Do,
                                queue_num=gno % NQ,
                            )
                            st_ = spool.tile([P, DGW * P], bf16, tag="s")
                            nc.vector.tensor_tensor(
                                out=st_[:, :Wg * P].rearrange("p (a b) -> p a b", a=Wg),
                                in0=dstl_sb[:, gc0:gc0 + Wg].to_broadcast([P, Wg, P]),
                                in1=iota_sb[:, :Wg * P].rearrange("p (a b) -> p a b", a=Wg),
                                op=mybir.AluOpType.is_equal,
                            )
                            for j in range(Wg):
                                c = gc0 + j
                                nt = min(P, NSH - t * P)
                                if col_first[c]:
                                    ps = psa.tile([P, D_H], mybir.dt.float32, space="PSUM", tag="psa")
                                nc.tensor.matmul(
                                    ps[:, :Do],
                                    lhsT=st_[:, j * P:(j + 1) * P],
                                    rhs=gt_[:, j * Do:(j + 1) * Do],
                                    start=bool(col_first[c]),
                                    stop=bool(col_last[c]),
                                )
                                if col_last[c]:
                                    _evac(t, nt, ps)
                    BN = max(GB, SB)
                    for b0 in ([] if DG else range(0, NCH_L, BN)):
                        nb = min(BN, NCH_L - b0)
                        # gather batch
                        gt_ = gpool.tile([P, BN * D_H], bf16, tag="g")
                        for j0 in range(0, nb, GB):
                            jn = min(GB, nb - j0)
                            nc.gpsimd.indirect_dma_start(
                                out=gt_[:, j0 * Do:(j0 + jn) * Do],
                                out_offset=None,
                                in_=agview[:],
                                in_offset=bass.IndirectOffsetOnAxis(
                                    ap=gidx_sb[:, b0 + j0:b0 + j0 + jn], axis=0
                                ),
                            )
                        # S batch
                        st_ = spool.tile([P, BN * P], bf16, tag="s")
                        for j0 in range(0, nb, SB):
                            jn = min(SB, nb - j0)
                            nc.vector.tensor_tensor(
                                out=st_[:, j0 * P:(j0 + jn) * P].rearrange(
                                    "p (a b) -> p a b", a=jn
                                ),
                                in0=dstl_sb[:, b0 + j0:b0 + j0 + jn].to_broadcast(
                                    [P, jn, P]
                                ),
                                in1=iota_sb[:, :jn * P].rearrange("p (a b) -> p a b", a=jn),
                                op=mybir.AluOpType.is_equal,
                            )
                        for j in range(nb):
                            c = b0 + j
                            t = int(col_tile[c])
                            nt = min(P, NSH - t * P)
                            if col_first[c]:
                                ps = psa.tile([P, D_H], mybir.dt.float32, space="PSUM", tag="psa")
                            nc.tensor.matmul(
                                ps[:, :Do],
                                lhsT=st_[:, j * P:(j + 1) * P],
                                rhs=gt_[:, j * Do:(j + 1) * Do],
                                start=bool(col_first[c]),
                                stop=bool(col_last[c]),
                            )
                            if col_last[c]:
                                _evac(t, nt, ps)

            # ---- heads ----
            with nc.named_scope("heads"):
                for t in range(TL if NCONVS >= 3 else 0):
                    nt = min(P, NSH - t * P)
                    ps = pst.tile([P, 72], mybir.dt.float32, space="PSUM", tag="tp")
                    for k in range(2):
                        nc.tensor.matmul(
                            ps[:, :],
                            lhsT=embT[:, k * NSHD + t * P: k * NSHD + (t + 1) * P],
                            rhs=wall[:, k * 72:(k + 1) * 72],
                            start=(k == 0),
                            stop=(k == 1),
                        )
                    hh = hpool.tile([P, 72], mybir.dt.float32, tag="hh")
                    nc.vector.tensor_tensor(
                        out=hh[:, :], in0=ps[:, :], in1=ballr[:, :],
                        op=mybir.AluOpType.add,
                    )
                    nc.sync.dma_start(heads_out[t * P:t * P + nt, :], hh[:nt, :])

    nc.compile()
    return nc


_CACHE = {}


def _run(inputs, trace=False):
    from concourse.bass_utils import run_bass_kernel_spmd

    x = np.asarray(inputs["x"], np.float32)
    edge_index = np.asarray(inputs["edge_index"])
    args = {k: np.asarray(inputs[k], np.float32) for k in
            ["W1", "b1", "W2", "b2", "W3", "b3", "We", "be", "Wh", "bh", "Wg", "bg"]}

    shared, per_core, C_t, NCH = _prep(x, edge_index, **args)

    key = ("prog", NCH, tuple(np.asarray(C_t).reshape(-1).tolist()))
    if key not in _CACHE:
        _CACHE[key] = _build(C_t, NCH)
    nc = _CACHE[key]

    in_maps = [{**shared, **pc} for pc in per_core]
    global _LAST
    _LAST = (nc, in_maps)
    res = run_bass_kernel_spmd(nc, in_maps, core_ids=list(range(NCORES)), trace=trace)

    emb = np.concatenate([res.results[c]["emb"] for c in range(NCORES)], axis=0)
    heads = np.concatenate([res.results[c]["heads"] for c in range(NCORES)], axis=0)
    out = (emb, heads[:, :7], heads[:, 7:15], heads[:, 15:])
    return out, res


def kernel(**inputs):
    out, _ = _run(inputs, trace=False)
    return out
